# revision 12
# baseline (speedup 1.0000x reference)
"""Trainium2 Bass kernel for nn_MiniMHCLM (moe_routing).

Strategy (8 NeuronCores, SPMD, no collectives):
  - vocab-sharded head matmul: core i holds w_head rows [i*VS:(i+1)*VS]
    (host-sliced, zero-padded to uniform VS) and computes logits for all
    4096 tokens x its vocab slice.  Host concatenates along vocab.
  - the cheap per-token pipeline (embed gather, RMS+phi coeffs, Sinkhorn,
    gather/scatter mixing) is replicated on every core.
  - embedding table is staged host-side in bf16 (numerically identical to
    the reference's `embed[ids].astype(bf16)`), gathered on-device via
    indirect DMA.
  - w_head slice is converted to bf16 on device, bounced through DRAM and
    loaded transposed (k-major) via the DMA xbar transpose; it then stays
    resident in SBUF for the whole kernel.
  - head matmul in bf16 with fp32 PSUM accumulation; PSUM evacuated by
    ACT/DVE copies (DMA cannot read PSUM) and DMA'd to DRAM fp32.
"""

import math
import numpy as np

HC, C, TMAX = 4, 256, 8
RMS_EPS, PRE_EPS, SINK_EPS, POST_MULT = 1e-6, 1e-4, 1e-6, 2.0
VOCAB = 50257
B, S = 2, 2048
K = HC * C            # 1024
M = HC * HC + 2 * HC  # 24
NKC = K // 128        # 8 k-chunks
NCORES = 8


class _Cfg:
    def __init__(self, vocab, vs, vpad, nt, g, nsc):
        self.vocab = vocab      # rows in the (full) embedding table
        self.vs = vs            # vocab-slice rows per core (uniform, padded)
        self.vpad = vpad        # vs padded for transpose/head loop
        self.nt = nt            # total tokens
        self.g = g              # 128-token chunks per super-chunk
        self.nsc = nsc          # super-chunks; nt == 128*g*nsc
        self.vw = min(512, vpad)
        assert vpad % self.vw == 0 and vpad % 16 == 0
        self.nv = vpad // self.vw
        assert nt == 128 * g * nsc


REAL = _Cfg(vocab=VOCAB, vs=6283, vpad=6656, nt=B * S, g=16, nsc=2)


def _build(cfg: _Cfg):
    from contextlib import ExitStack
    from concourse import bass, bacc, mybir
    import concourse.tile as tile
    from concourse.masks import make_identity

    f32 = mybir.dt.float32
    bf16 = mybir.dt.bfloat16
    i32 = mybir.dt.int32
    AX = mybir.AxisListType
    OP = mybir.AluOpType
    AF = mybir.ActivationFunctionType

    nt, g, nsc, vs, vpad, vw, nv = (
        cfg.nt, cfg.g, cfg.nsc, cfg.vs, cfg.vpad, cfg.vw, cfg.nv)
    nchunks = nt // 128
    g4, g16 = g * 4, g * 16

    nc = bacc.Bacc(target_bir_lowering=False)
    ids_p = nc.declare_dram_parameter("ids", [128, nchunks], i32, False)
    emb_p = nc.declare_dram_parameter("emb", [cfg.vocab, K], bf16, False)
    wv_p = nc.declare_dram_parameter("wv", [vs, K], f32, False)
    wi_p = nc.declare_dram_parameter("wi", [C, C], f32, False)
    phi_p = nc.declare_dram_parameter("phi", [K, M], bf16, False)
    b_p = nc.declare_dram_parameter("b", [1, M], f32, False)
    al_p = nc.declare_dram_parameter("al", [1, 3], f32, False)
    out_p = nc.declare_dram_parameter("out", [nt, vs], f32, True)

    with ExitStack() as ctx:
        tc = ctx.enter_context(tile.TileContext(nc))
        dram = ctx.enter_context(tc.tile_pool(name="dram", bufs=1, space="DRAM"))
        const = ctx.enter_context(tc.tile_pool(name="const", bufs=1))
        wtp = ctx.enter_context(tc.tile_pool(name="wtp", bufs=1))
        xbfp = ctx.enter_context(tc.tile_pool(name="xbfp", bufs=1))
        scp = ctx.enter_context(tc.tile_pool(name="scp", bufs=2))     # per-SC coeff bufs
        wk = ctx.enter_context(tc.tile_pool(name="wk", bufs=2))       # small working tiles
        wk3 = ctx.enter_context(tc.tile_pool(name="wk3", bufs=3))
        outp = ctx.enter_context(tc.tile_pool(name="outp", bufs=6))
        pst = ctx.enter_context(tc.tile_pool(name="pst", bufs=2, space="PSUM"))
        psc = ctx.enter_context(tc.tile_pool(name="psc", bufs=2, space="PSUM"))
        psh = ctx.enter_context(tc.tile_pool(name="psh", bufs=4, space="PSUM"))

        # ---------------- prep ----------------
        ident = const.tile([128, 128], bf16)
        make_identity(nc, ident[:])

        cst = const.tile([128, 2], f32)
        nc.vector.memset(cst[:, 0:1], 0.0)
        nc.vector.memset(cst[:, 1:2], RMS_EPS)
        zero_b = cst[:, 0:1]
        eps_b = cst[:, 1:2]

        phi_sb = const.tile([128, NKC * M], bf16)
        for kc in range(NKC):
            nc.sync.dma_start(out=phi_sb[:, kc * M:(kc + 1) * M],
                              in_=phi_p[kc * 128:(kc + 1) * 128, :])
        # broadcast b/alpha row to all 128 partitions via stride-0 DMA reads
        b_bc = const.tile([128, M], f32)
        nc.sync.dma_start(out=b_bc[:], in_=b_p[0:1, :].to_broadcast([128, M]))
        al_bc = const.tile([128, 3], f32)
        nc.sync.dma_start(out=al_bc[:], in_=al_p[0:1, :].to_broadcast([128, 3]))

        ids_all = const.tile([128, nchunks], i32)
        nc.sync.dma_start(out=ids_all[:], in_=ids_p[:, :])

        # w_inner -> w_iT (bf16, k-major): w_iT[:, ib*C + o] rows = i-chunk ib
        w_iT = const.tile([128, 2 * C], bf16)
        pt0 = pst.tile([128, 1024], bf16, tag="pst")
        for ob in range(2):
            wif = wk.tile([128, C], f32, tag="wif")
            nc.sync.dma_start(out=wif[:], in_=wi_p[ob * 128:(ob + 1) * 128, :])
            wib = wk.tile([128, C], bf16, tag="wib")
            nc.scalar.copy(wib[:], wif[:])
            for ib in range(2):
                nc.tensor.transpose(
                    out=pt0[:, ib * C + ob * 128: ib * C + (ob + 1) * 128],
                    in_=wib[:, ib * 128:(ib + 1) * 128],
                    identity=ident[:])
        nc.scalar.copy(w_iT[:], pt0[:, :2 * C])

        # w_head slice -> bf16, transposed k-major into resident SBUF tiles
        # (PE-transpose path; all wt writes on ACT so consumers have
        # single-engine deps)
        zt = const.tile([128, K], bf16)
        nc.vector.memset(zt[:], 0.0)
        wt = []
        for kc in range(NKC):
            t = wtp.tile([128, vpad], bf16, tag=f"wt{kc}")
            wt.append(t)
        for r in range(0, vpad, 128):
            rows = min(128, vpad - r)
            vrows = min(max(vs - r, 0), rows)
            wbr = wk3.tile([128, K], bf16, tag="wbr")
            if vrows < rows:
                nc.scalar.copy(wbr[:], zt[:])
            if vrows > 0:
                wfr = wk3.tile([128, K], f32, tag="wfr")
                nc.sync.dma_start(out=wfr[:vrows], in_=wv_p[r:r + vrows, :])
                nc.scalar.copy(wbr[:vrows], wfr[:vrows])
            ptw = pst.tile([128, 1024], bf16, tag="pst")
            for kc in range(NKC):
                nc.tensor.transpose(
                    out=ptw[:, kc * 128:(kc + 1) * 128],
                    in_=wbr[:, kc * 128:(kc + 1) * 128], identity=ident[:])
            for kc in range(NKC):
                nc.scalar.copy(wt[kc][:, r:r + rows],
                               ptw[:, kc * 128:kc * 128 + rows])

        # ---------------- main ----------------
        for sc in range(nsc):
            m_all = scp.tile([128, g16], f32, tag="m_all")
            hpre = scp.tile([128, g4], f32, tag="hpre")
            hpost2 = scp.tile([128, g4], f32, tag="hpost2")
            scl = scp.tile([128, g], f32, tag="scl")
            xbs = []

            # ---- pass A: gather + coeff logits ----
            for c in range(g):
                cc = sc * g + c
                xb = xbfp.tile([128, K], bf16, tag=f"xb{c}")
                nc.gpsimd.indirect_dma_start(
                    out=xb[:], out_offset=None,
                    in_=emb_p[:, :],
                    in_offset=bass.IndirectOffsetOnAxis(
                        ap=ids_all[:, cc:cc + 1], axis=0))
                xbs.append(xb)

                # RMS scale
                dump = wk.tile([128, K], bf16, tag="dump")
                ssq = wk.tile([128, 1], f32, tag="ssq")
                nc.scalar.activation(out=dump[:], in_=xb[:], func=AF.Square,
                                     bias=zero_b, accum_out=ssq[:])
                srt = wk.tile([128, 1], f32, tag="srt")
                nc.scalar.activation(out=srt[:], in_=ssq[:], func=AF.Sqrt,
                                     scale=1.0 / K, bias=eps_b)
                nc.vector.reciprocal(scl[:, c:c + 1], srt[:])

                # x^T (k-major) for the phi matmul
                ptx = pst.tile([128, 1024], bf16, tag="pst")
                for kc in range(NKC):
                    nc.tensor.transpose(
                        out=ptx[:, kc * 128:(kc + 1) * 128],
                        in_=xb[:, kc * 128:(kc + 1) * 128], identity=ident[:])
                xT = wk.tile([128, 1024], bf16, tag="xT")
                nc.scalar.copy(xT[:], ptx[:])

                pc = psc.tile([128, C], f32, tag="psc")
                for kc in range(NKC):
                    nc.tensor.matmul(
                        out=pc[:, :M],
                        lhsT=xT[:, kc * 128:(kc + 1) * 128],
                        rhs=phi_sb[:, kc * M:(kc + 1) * M],
                        start=(kc == 0), stop=(kc == NKC - 1))
                lg = wk.tile([128, M], f32, tag="lg")
                nc.vector.tensor_scalar_mul(lg[:], pc[:, :M], scl[:, c:c + 1])
                nc.vector.tensor_add(lg[:], lg[:], b_bc[:])

                nc.scalar.activation(out=hpre[:, c * 4:(c + 1) * 4],
                                     in_=lg[:, 0:4], func=AF.Sigmoid,
                                     bias=zero_b, scale=al_bc[:, 0:1])
                nc.scalar.activation(out=hpost2[:, c * 4:(c + 1) * 4],
                                     in_=lg[:, 4:8], func=AF.Sigmoid,
                                     bias=zero_b, scale=al_bc[:, 1:2])
                nc.scalar.activation(out=m_all[:, c * 16:(c + 1) * 16],
                                     in_=lg[:, 8:24], func=AF.Exp,
                                     bias=zero_b, scale=al_bc[:, 2:3])
            nc.vector.tensor_scalar_add(hpre[:], hpre[:], PRE_EPS)
            nc.vector.tensor_scalar_mul(hpost2[:], hpost2[:], POST_MULT)

            # ---- pass B: batched Sinkhorn ----
            mv3 = m_all[:].rearrange("p (a i) -> p a i", i=4)
            mv4 = m_all[:].rearrange("p (g o i) -> p g o i", o=4, i=4)
            mv4t = m_all[:].rearrange("p (g o i) -> p g i o", o=4, i=4)
            for _ in range(TMAX):
                rs = wk.tile([128, g4], f32, tag="rs")
                nc.vector.tensor_reduce(rs[:], mv3, axis=AX.X, op=OP.add)
                nc.vector.tensor_scalar_add(rs[:], rs[:], SINK_EPS)
                nc.vector.reciprocal(rs[:], rs[:])
                nc.vector.tensor_tensor(
                    out=mv3, in0=mv3,
                    in1=rs[:][:, :, None].to_broadcast([128, g4, 4]),
                    op=OP.mult)
                cs = wk.tile([128, g4], f32, tag="cs")
                nc.vector.tensor_reduce(cs[:], mv4t, axis=AX.X, op=OP.add)
                nc.vector.tensor_scalar_add(cs[:], cs[:], SINK_EPS)
                nc.vector.reciprocal(cs[:], cs[:])
                nc.vector.tensor_tensor(
                    out=mv4, in0=mv4,
                    in1=cs[:].rearrange("p (g i) -> p g i", i=4)
                         [:, :, None, :].to_broadcast([128, g, 4, 4]),
                    op=OP.mult)

            # ---- pass C: mixing + head matmul ----
            for c in range(g):
                cc = sc * g + c
                t0 = cc * 128
                xb = xbs[c]

                # x_in = sum_i h_pre[i] * x[i]
                xin = wk.tile([128, C], bf16, tag="xin")
                tmp = wk3.tile([128, C], bf16, tag="tmp")
                nc.vector.tensor_scalar_mul(
                    xin[:], xb[:, 0:C], hpre[:, c * 4:c * 4 + 1])
                for i in range(1, HC):
                    tmp = wk3.tile([128, C], bf16, tag="tmp")
                    nc.vector.tensor_scalar_mul(
                        tmp[:], xb[:, i * C:(i + 1) * C],
                        hpre[:, c * 4 + i:c * 4 + i + 1])
                    nc.vector.tensor_add(xin[:], xin[:], tmp[:])
                # x_in^T
                pti = pst.tile([128, 1024], bf16, tag="pst")
                for ib in range(2):
                    nc.tensor.transpose(
                        out=pti[:, ib * 128:(ib + 1) * 128],
                        in_=xin[:, ib * 128:(ib + 1) * 128], identity=ident[:])
                xiT = wk.tile([128, C], bf16, tag="xiT")
                nc.scalar.copy(xiT[:], pti[:, :C])
                # f_out = x_in @ w_inner.T
                pf = psc.tile([128, C], f32, tag="psc")
                for ib in range(2):
                    nc.tensor.matmul(
                        out=pf[:], lhsT=xiT[:, ib * 128:(ib + 1) * 128],
                        rhs=w_iT[:, ib * C:(ib + 1) * C],
                        start=(ib == 0), stop=(ib == 1))
                fo = wk.tile([128, C], bf16, tag="fo")
                nc.scalar.copy(fo[:], pf[:])

                # x_merge[o] = sum_i h_res[o,i]*x[i] + h_post2[o]*f_out
                xmg = wk.tile([128, K], bf16, tag="xmg")
                for o in range(HC):
                    seg = xmg[:, o * C:(o + 1) * C]
                    base = c * 16 + o * 4
                    nc.vector.tensor_scalar_mul(
                        seg, xb[:, 0:C], m_all[:, base:base + 1])
                    for i in range(1, HC):
                        tmp = wk3.tile([128, C], bf16, tag="tmp")
                        nc.vector.tensor_scalar_mul(
                            tmp[:], xb[:, i * C:(i + 1) * C],
                            m_all[:, base + i:base + i + 1])
                        nc.vector.tensor_add(seg, seg, tmp[:])
                    tmp = wk3.tile([128, C], bf16, tag="tmp")
                    nc.vector.tensor_scalar_mul(
                        tmp[:], fo[:], hpost2[:, c * 4 + o:c * 4 + o + 1])
                    nc.vector.tensor_add(seg, seg, tmp[:])

                # x_merge^T (k-major)
                ptm = pst.tile([128, 1024], bf16, tag="pst")
                for kc in range(NKC):
                    nc.tensor.transpose(
                        out=ptm[:, kc * 128:(kc + 1) * 128],
                        in_=xmg[:, kc * 128:(kc + 1) * 128], identity=ident[:])
                xmT = wk.tile([128, 1024], bf16, tag="xmT")
                nc.scalar.copy(xmT[:], ptm[:])

                # head matmul over the vocab slice
                for v in range(nv):
                    ph = psh.tile([128, vw], f32, tag="psh")
                    for kc in range(NKC):
                        nc.tensor.matmul(
                            out=ph[:],
                            lhsT=xmT[:, kc * 128:(kc + 1) * 128],
                            rhs=wt[kc][:, v * vw:(v + 1) * vw],
                            start=(kc == 0), stop=(kc == NKC - 1))
                    st = outp.tile([128, vw], f32, tag="st")
                    if v % 2 == 0:
                        nc.scalar.copy(st[:], ph[:])
                    else:
                        nc.vector.tensor_copy(st[:], ph[:])
                    w = min(vw, vs - v * vw)
                    if w > 0:
                        nc.sync.dma_start(
                            out=out_p[t0:t0 + 128, v * vw:v * vw + w],
                            in_=st[:, :w])
    if not nc.is_finalized():
        nc.finalize()
    return nc


_NC_CACHE = {}


def _get_nc(cfg):
    key = (cfg.vocab, cfg.vs, cfg.vpad, cfg.nt, cfg.g, cfg.nsc)
    if key not in _NC_CACHE:
        _NC_CACHE[key] = _build(cfg)
    return _NC_CACHE[key]


def _make_in_maps(cfg, input_ids, embed, w_inner, w_head, phi, b,
                  alpha_pre, alpha_post, alpha_res):
    import ml_dtypes
    bf = ml_dtypes.bfloat16
    nt, vs = cfg.nt, cfg.vs

    ids = np.ascontiguousarray(
        np.asarray(input_ids).astype(np.int32).reshape(-1, 128).T)  # [128, nchunks]
    emb = np.asarray(embed).astype(bf)
    phi_np = np.asarray(phi).astype(bf)
    wi = np.ascontiguousarray(np.asarray(w_inner, dtype=np.float32))
    b_np = np.asarray(b, dtype=np.float32).reshape(1, M)
    al = np.array([[np.asarray(alpha_pre).reshape(-1)[0],
                    np.asarray(alpha_post).reshape(-1)[0],
                    np.asarray(alpha_res).reshape(-1)[0]]], dtype=np.float32)
    wh = np.asarray(w_head, dtype=np.float32)

    in_maps = []
    for i in range(NCORES):
        sl = wh[i * vs:(i + 1) * vs]
        if sl.shape[0] < vs:
            sl = np.concatenate(
                [sl, np.zeros((vs - sl.shape[0], sl.shape[1]), np.float32)], 0)
        in_maps.append(dict(ids=ids, emb=emb, wv=np.ascontiguousarray(sl),
                            wi=wi, phi=phi_np, b=b_np, al=al))
    return in_maps


def _run(cfg, in_maps, trace=False):
    from concourse.bass_utils import run_bass_kernel_spmd
    nc = _get_nc(cfg)
    return run_bass_kernel_spmd(nc, in_maps, list(range(NCORES)), trace=trace)


def kernel(input_ids, embed, w_inner, w_head, phi, b,
           alpha_pre, alpha_post, alpha_res):
    cfg = REAL
    in_maps = _make_in_maps(cfg, input_ids, embed, w_inner, w_head, phi, b,
                            alpha_pre, alpha_post, alpha_res)
    res = _run(cfg, in_maps).results
    out = np.concatenate([np.asarray(res[i]["out"]) for i in range(NCORES)],
                         axis=1)[:, :VOCAB]
    return np.ascontiguousarray(out.reshape(B, S, VOCAB).astype(np.float32))
